# revision 29
# baseline (speedup 1.0000x reference)
"""MinGRU forward on 8 TRN2 NeuronCores.

Math (linear-space reformulation of the reference's log-space Heinsen scan):
    hg = x @ W_hg.T                       # [B,S,2D]
    hidden, gate = split(hg)
    z = sigmoid(gate)
    c = sigmoid(-gate)                    # = 1 - z = exp(-softplus(gate))
    g = max(hidden + 0.5, sigmoid(hidden))  # == where(h>=0, h+0.5, sigmoid(h)) exactly
    u = z * g
    h[t] = c[t] * h[t-1] + u[t]           # convex combination -> bounded, stable
    out = h

The recurrence maps onto the DVE `tensor_tensor_scan` instruction
(state = data0*state + data1 along the free dim, fp32 internal state).

Sharding: 8 cores = 4 batches x 2 feature-halves (512 features each); the
scan is per-feature independent so there is no cross-core communication.

Design notes (each measured on HW):
- fp16 operands: matmuls stream at ~217ns/512 cols (vs fp32r 227) and the
  HBM-bound opening halves.  The old "fp16 nets 259ns" result was a DMA
  artifact of thin-packet layouts.  rel err 2.2e-3 (10-bit mantissa).
- Host lays out W [fc][p][k][2FC] and x chunk-major [p][k][w] -- the exact
  SBUF tile order -- so every DMA moves 2-8KB contiguous runs/partition.
  The rings arbitrate HBM per packet; thin-packet streams get starved.
- Pair-blocked unit order ((sc-pair x fc-pair) blocks) halves the W
  working set the opening must deliver before the PE streams gap-free;
  gaps also reset the HAM clock-ramp (full speed needs ~3us CONTINUOUS
  PE activity).
- warm_ldw standalone 2-byte ldweights (fp32/fp32r standalone ldweights
  are illegal) pre-open the clock gate during the preamble/opening-DMA
  window.  72 is tuned so the slightly-ramped PE PACES the opening DMA
  stream; a fully-warm PE outpaces it and starves (-6us).
- Last chunk: gate matmul group runs BEFORE hidden, so the z/c sigmoids
  compute during the hidden matmuls, cutting ~2.5us off the tail chain.
"""

import numpy as np

B, S, D = 4, 4096, 1024
DH = D // 2          # features per core
N_CORES = 8
SC = 512             # tokens per seq chunk (PSUM bank = 512 fp32)
KC = 128             # contraction chunk
NKC = D // KC        # 8 k chunks
FC = 128             # feature chunk (psum partitions)
NFC = DH // FC       # 4 feature chunks

_CACHE = {}

CONFIG = {
    "xbufs": 4,            # pair-blocking: x tiles live across 2 fc-blocks
    "psbufs": 4,
    "ebufs": 3,
    "split_last_scan": True,
    "warm_ldw": 72,  # last chunk: 2 chained half-scans so out-DMA overlaps
}

WIDTHS = [512, 512, 512, 512, 512, 512, 512, 512]
assert sum(WIDTHS) == S


def _unit_order(n_sc):
    """(sc, fc) units in (sc-pair x fc-pair) blocks.

    Block = 4 units sharing 2 W tiles + 2 x chunks; halves the W working
    set the opening must deliver before the PE can run uninterrupted.
    """
    order = []
    scps = [(a, a + 1) if a + 1 < n_sc else (a,) for a in range(0, n_sc, 2)]
    for scp in scps:
        for fcp in ((0, 1), (2, 3)):
            for sc in scp:
                for fc in fcp:
                    order.append((sc, fc))
    return order


def _build():
    import concourse.bacc as bacc
    import concourse.tile as tile
    import concourse.mybir as mybir

    f32 = mybir.dt.float32
    fmm = mybir.dt.float16    # matmul operand dtype (10-bit mantissa ~ fp32r's 11)
    AF = mybir.ActivationFunctionType
    OP = mybir.AluOpType

    nc = bacc.Bacc("TRN2")
    # x stored chunk-major in SBUF tile order: chunk sc occupies the flat
    # range [offs[sc]*D, offs[sc+1]*D) laid out [p][k][w], so a chunk DMA
    # moves NKC*width*4B (up to 16KB) contiguous runs per partition and a
    # k-granule 4-8KB runs.  The DMA rings arbitrate HBM per PACKET, so
    # bandwidth share is proportional to packet size: with the old [D, S]
    # layout x's 2KB packets got starved ~2:1 by W's 4-8KB packets and the
    # first matmul waited on x until ~14us.
    xT = nc.dram_tensor("xT", [D * S], fmm, kind="ExternalInput")
    # wT layout matches the SBUF tiles exactly: [fc][partition][k][2*FC]
    # (128 hidden cols then 128 gate cols).  Per partition a k-half is 4KB
    # contiguous -> 4096-byte DMA descriptor elements; the rings are
    # packet-rate limited, so bytes/packet sets ring throughput (~2x the
    # old [D, NFC, 2*FC] layout's 1KB elements).
    wT = nc.dram_tensor("wT", [NFC, KC, NKC, 2 * FC], fmm, kind="ExternalInput")
    outT = nc.dram_tensor("outT", [DH, S], f32, kind="ExternalOutput")

    with tile.TileContext(nc) as tc:
        with (
            tc.tile_pool(name="w", bufs=1) as wpool,
            tc.tile_pool(name="x", bufs=CONFIG["xbufs"]) as xpool,
            tc.tile_pool(name="ew", bufs=CONFIG["ebufs"]) as epool,
            tc.tile_pool(name="h", bufs=2) as hpool,
            tc.tile_pool(name="ps", bufs=CONFIG["psbufs"], space="PSUM") as pspool,
        ):
            wts = []
            for fc in range(NFC):
                wtf = wpool.tile([KC, NKC, 2 * FC], fmm, tag=f"w{fc}")
                wts.append(wtf)

            # PE clock-gate warmup: HAM holds the PE at ~50% until ~3us of
            # CONTINUOUS activity; idle re-arms it.  Standalone 2-byte
            # ldweights (legal, unlike fp32/fp32r) busy the PE from ~7.5us
            # through first-data (~12us) so real matmuls start at speed.
            if CONFIG.get("warm_ldw"):
                wrm = wpool.tile([KC, FC], fmm, tag="wrm")
                nc.gpsimd.memset(wrm[:], 0.0)
                for _ in range(CONFIG["warm_ldw"]):
                    nc.tensor.ldweights(wrm[:])

            offs = np.concatenate([[0], np.cumsum(WIDTHS)]).astype(int)

            # Opening DMAs.  W (4KB elems) on the scalar/ACT ring, fc0
            # first (its first half gates the first matmul), fc2/fc3 whole
            # so their descriptors trail fc1 and don't contend with x
            # while sc0/sc1 stream; x (2KB elems) on the sync ring in
            # k-granules that the k-interleaved sc0/sc1 matmuls chase,
            # plus x2/x3 prefetch (needed only at the second sc-pair,
            # +29us, but the ring is free once x0/x1 land).  W DMAs are
            # issued first so the HWDGE semaphore-lane recycling (9 lanes)
            # puts the false waits on late, slack-rich x DMAs.
            nc.scalar.dma_start(wts[0][:, 0:2, :], wT[0, :, 0:2, :])
            nc.scalar.dma_start(wts[0][:, 2:4, :], wT[0, :, 2:4, :])
            nc.scalar.dma_start(wts[0][:, 4:8, :], wT[0, :, 4:8, :])
            nc.scalar.dma_start(wts[1][:, 0:4, :], wT[1, :, 0:4, :])
            nc.scalar.dma_start(wts[1][:, 4:8, :], wT[1, :, 4:8, :])
            nc.scalar.dma_start(wts[2][:], wT[2, :, :, :])
            nc.scalar.dma_start(wts[3][:], wT[3, :, :, :])

            def _load_x(sc, granules=None):
                width = WIDTHS[sc]
                base = int(offs[sc]) * D
                xt = xpool.tile([KC, NKC, width], fmm, tag="xt")
                xT_r = xT[base:base + width * D].rearrange(
                    "(p k w) -> p k w", p=KC, k=NKC
                )
                if granules is None:
                    nc.sync.dma_start(xt[:], xT_r)
                else:
                    for a, b in granules:
                        nc.sync.dma_start(xt[:, a:b, :], xT_r[:, a:b, :])
                return xt

            xts = {}
            xts[0] = _load_x(0, granules=[(0, 2), (2, 4), (4, 8)])
            xts[1] = _load_x(1, granules=[(0, 4), (4, 8)])
            xts[2] = _load_x(2)
            xts[3] = _load_x(3)

            order = _unit_order(len(WIDTHS))

            hprev = [None] * NFC
            for ui, (sc, fc) in enumerate(order):
                width = WIDTHS[sc]
                off = int(offs[sc])
                # at each sc-pair block start, prefetch the NEXT pair's x
                if ui > 0 and ui % 8 == 0:
                    for scn in (sc + 2, sc + 3):
                        if scn < len(WIDTHS) and scn not in xts:
                            xts[scn] = _load_x(scn)
                xt = xts[sc]
                ph = pspool.tile([FC, width], f32, tag="ph")
                pg = pspool.tile([FC, width], f32, tag="pg")
                if sc <= 1:
                    # interleave h/g per k-slice: each newly-landed slice
                    # feeds 2 matmuls, halving PE stalls while DMA-gated
                    for k in range(NKC):
                        nc.tensor.matmul(
                            ph[:], wts[fc][:, k, 0:FC], xt[:, k, :],
                            start=(k == 0), stop=(k == NKC - 1),
                        )
                        nc.tensor.matmul(
                            pg[:], wts[fc][:, k, FC:2 * FC], xt[:, k, :],
                            start=(k == 0), stop=(k == NKC - 1),
                        )
                elif sc == len(WIDTHS) - 1:
                    for k in range(NKC):
                        nc.tensor.matmul(
                            pg[:], wts[fc][:, k, FC:2 * FC], xt[:, k, :],
                            start=(k == 0), stop=(k == NKC - 1),
                        )
                    for k in range(NKC):
                        nc.tensor.matmul(
                            ph[:], wts[fc][:, k, 0:FC], xt[:, k, :],
                            start=(k == 0), stop=(k == NKC - 1),
                        )
                else:
                    for k in range(NKC):
                        nc.tensor.matmul(
                            ph[:], wts[fc][:, k, 0:FC], xt[:, k, :],
                            start=(k == 0), stop=(k == NKC - 1),
                        )
                    for k in range(NKC):
                        nc.tensor.matmul(
                            pg[:], wts[fc][:, k, FC:2 * FC], xt[:, k, :],
                            start=(k == 0), stop=(k == NKC - 1),
                        )
                zt = epool.tile([FC, width], f32, tag="z")
                ct = epool.tile([FC, width], f32, tag="c")
                st = epool.tile([FC, width], f32, tag="s")
                gt = epool.tile([FC, width], f32, tag="g")
                ut = epool.tile([FC, width], f32, tag="u")
                if sc == len(WIDTHS) - 1:
                    # gate matmuls ran first: z/c compute during ph's group
                    nc.scalar.activation(zt[:], pg[:], AF.Sigmoid)
                    nc.scalar.activation(ct[:], pg[:], AF.Sigmoid, scale=-1.0)
                    nc.scalar.activation(st[:], ph[:], AF.Sigmoid)
                else:
                    # s first: it heads the DVE critical chain (s->g->u->scan)
                    nc.scalar.activation(st[:], ph[:], AF.Sigmoid)
                    nc.scalar.activation(zt[:], pg[:], AF.Sigmoid)
                    nc.scalar.activation(ct[:], pg[:], AF.Sigmoid, scale=-1.0)
                # g = (hidden + 0.5) max sigmoid(hidden)
                nc.vector.scalar_tensor_tensor(
                    gt[:], ph[:], 0.5, st[:], op0=OP.add, op1=OP.max
                )
                nc.vector.tensor_mul(ut[:], zt[:], gt[:])
                ht = hpool.tile([FC, width], f32, tag=f"h{fc}")
                pw = WIDTHS[sc - 1]
                init = 0.0 if sc == 0 else hprev[fc][:, pw - 1:pw]
                if CONFIG["split_last_scan"] and sc == len(WIDTHS) - 1:
                    hw_ = width // 2
                    nc.vector.tensor_tensor_scan(
                        ht[:, 0:hw_], ct[:, 0:hw_], ut[:, 0:hw_], init,
                        op0=OP.mult, op1=OP.add,
                    )
                    nc.sync.dma_start(
                        outT[fc * FC:(fc + 1) * FC, off:off + hw_], ht[:, 0:hw_]
                    )
                    nc.vector.tensor_tensor_scan(
                        ht[:, hw_:width], ct[:, hw_:width], ut[:, hw_:width],
                        ht[:, hw_ - 1:hw_], op0=OP.mult, op1=OP.add,
                    )
                    nc.sync.dma_start(
                        outT[fc * FC:(fc + 1) * FC, off + hw_:off + width],
                        ht[:, hw_:width],
                    )
                    hprev[fc] = ht
                else:
                    nc.vector.tensor_tensor_scan(
                        ht[:], ct[:], ut[:], init, op0=OP.mult, op1=OP.add
                    )
                    hprev[fc] = ht
                    nc.sync.dma_start(
                        outT[fc * FC:(fc + 1) * FC, off:off + width], ht[:]
                    )

    nc.compile()
    return nc


def _round_fp32r(a: np.ndarray) -> np.ndarray:
    """Round fp32 array to fp32r (11 explicit mantissa bits) with RNE."""
    u = np.ascontiguousarray(a, dtype=np.float32).view(np.uint32)
    r = (u + np.uint32(0x7FF) + ((u >> np.uint32(12)) & np.uint32(1))) & np.uint32(0xFFFFF000)
    return r.view(np.float32)


def _prep_in_maps(x: np.ndarray, W_hg: np.ndarray):
    x = np.asarray(x, dtype=np.float32)
    W_hg = np.asarray(W_hg, dtype=np.float32)
    offs = np.concatenate([[0], np.cumsum(WIDTHS)]).astype(int)
    xTs = []
    for b in range(B):
        xb = x[b].T                                   # [D, S]
        # chunk-major flat layout: per chunk [p][k][w] (d = k*KC + p)
        flat = np.empty(D * S, dtype=np.float32)
        for sc, w in enumerate(WIDTHS):
            chunk = xb[:, offs[sc]:offs[sc] + w]      # [D, w]
            flat[offs[sc] * D:offs[sc + 1] * D] = (
                chunk.reshape(NKC, KC, w).transpose(1, 0, 2).reshape(-1)
            )
        xTs.append(flat.astype(np.float16))
    wTs = []
    for c in range(2):
        # [NFC, KC, NKC, 2*FC]: per fc, SBUF tile order [p][k][2*FC]
        wt = np.empty((NFC, KC, NKC, 2 * FC), dtype=np.float32)
        for fc in range(NFC):
            rows_h = W_hg[c * DH + fc * FC:c * DH + (fc + 1) * FC]      # [FC, D]
            rows_g = W_hg[D + c * DH + fc * FC:D + c * DH + (fc + 1) * FC]
            # [FC, D] -> [FC, NKC, KC] -> [KC, NKC, FC]
            wt[fc, :, :, 0:FC] = rows_h.reshape(FC, NKC, KC).transpose(2, 1, 0)
            wt[fc, :, :, FC:2 * FC] = rows_g.reshape(FC, NKC, KC).transpose(2, 1, 0)
        wTs.append(wt.astype(np.float16))
    return [{"xT": xTs[core // 2], "wT": wTs[core % 2]} for core in range(N_CORES)]


def _get_runner():
    """Build the Bass module once and cache a compiled jax callable for it.

    Mirrors bass2jax.run_bass_via_pjrt's multi-core path, but keeps the
    jitted/sharded executable so repeat kernel() calls skip re-tracing.
    """
    if "runner" in _CACHE:
        return _CACHE["runner"]

    import jax
    from jax.experimental.shard_map import shard_map
    from jax.sharding import Mesh, PartitionSpec
    from concourse import bass2jax

    if "nc" not in _CACHE:
        _CACHE["nc"] = _build()
    nc = _CACHE["nc"]
    bass2jax.install_neuronx_cc_hook()

    in_names = ["xT", "wT"]
    out_name = "outT"
    out_shape, out_dtype = (DH, S), np.float32
    partition_name = nc.partition_id_tensor.name if nc.partition_id_tensor else None

    def _body(xT, wT, zout):
        operands = [xT, wT, zout]
        if partition_name is not None:
            operands.append(bass2jax.partition_id_tensor())
        outs = bass2jax._bass_exec_p.bind(
            *operands,
            out_avals=(jax.core.ShapedArray(out_shape, out_dtype),),
            in_names=tuple(in_names + [out_name] + ([partition_name] if partition_name else [])),
            out_names=(out_name,),
            lowering_input_output_aliases=(),
            sim_require_finite=True,
            sim_require_nnan=True,
            nc=nc,
        )
        return tuple(outs)

    devices = jax.devices()[:N_CORES]
    mesh = Mesh(np.asarray(devices), ("core",))
    sharded = jax.jit(
        shard_map(
            _body, mesh=mesh,
            in_specs=(PartitionSpec("core"),) * 3,
            out_specs=(PartitionSpec("core"),),
            check_rep=False,
        ),
        donate_argnums=(2,),
        keep_unused=True,
    )

    def run(in_maps):
        concat_x = np.concatenate([m["xT"] for m in in_maps], axis=0)
        concat_w = np.concatenate([m["wT"] for m in in_maps], axis=0)
        zeros = np.zeros((N_CORES * DH, S), np.float32)
        (out_arr,) = sharded(concat_x, concat_w, zeros)
        return np.asarray(out_arr).reshape(N_CORES, DH, S)

    _CACHE["runner"] = run
    return run


def kernel(x: np.ndarray, W_hg: np.ndarray) -> np.ndarray:
    run = _get_runner()
    in_maps = _prep_in_maps(x, W_hg)
    outs = run(in_maps)

    out = np.empty((B, S, D), dtype=np.float32)
    for core in range(N_CORES):
        b, c = core // 2, core % 2
        out[b, :, c * DH:(c + 1) * DH] = outs[core].T
    return out
